# revision 3
# baseline (speedup 1.0000x reference)
"""Grouped SwiGLU expert FFN (MoE) on 8 Trainium2 NeuronCores.

Expert parallelism: expert e's weights + its (pre-sorted) token slice go to
core e. Each core runs x@w1, x@w3, silu/mul, h@w2 for its 8192 tokens.

v7: host-transposed x — no xbar transposes on device.
 - v5/v6 traces showed every normal<->transpose DMA transition gets a
   cross-queue completion fence (shared xbar mode), serializing the
   startup wave and pinning MM0 at 20-23 us.  The host now supplies
   x already transposed ([dim_in, tokens] fp16, contiguous), so every
   device DMA is a plain strided load and the fences vanish.
 - weights load as per-j 128-column chunks, interleaved across the
   gpsimd and scalar queues, so mm1/mm3's j-th stationary always lands
   well before the PE reaches it; w2 follows on gpsimd (needed ~37us).
 - fp16 output store (host upcasts); last block drains per-(t,o).

Math per core (dims: t=tokens, i=dim_in, j=dim_hid, o=dim_in):
  mm1/mm3: psum[j,t] += lhsT=w{1,3}[i_chunk, j_chunk] (stationary),
           rhs=xT[i_chunk, t_block] (moving 512) -> h1T/h3T.
  SwiGLU:  hT = silu(h1T) * h3T  (ACT Silu -> fp16, DVE mul -> fp16).
  mm2:     lhsT=hT[j_chunk, t_chunk] (stationary), rhs=w2[j_chunk, o_block]
           (moving 512) -> psum[t,o] natural-layout output -> fp16 store.
"""

import sys

sys.path.insert(0, "/opt/trn_rl_repo")

import numpy as np

N_CORES = 8
D = 1024  # dim_in
H = 1024  # dim_hid
P = 128
TB = 512  # token block per pipeline stage

_CACHE = {}


def _build(tok):
    import concourse.bacc as bacc
    import concourse.tile as tile
    from concourse import mybir

    dt = mybir.dt
    AF = mybir.ActivationFunctionType
    f32 = dt.float32
    f16 = dt.float16

    assert tok % TB == 0
    n_blk = tok // TB
    n_i = D // P   # 8 contraction chunks for mm1/mm3
    n_j = H // P   # 8 contraction chunks for mm2
    n_tc = TB // P  # 4 token chunks per block
    n_o = D // 512  # 2 output column blocks

    nc = bacc.Bacc(trn_type="TRN2", target_bir_lowering=False)
    xT_h = nc.dram_tensor("xT", [D, tok], f16, kind="ExternalInput")
    w1_h = nc.dram_tensor("w1", [D, H], f16, kind="ExternalInput")
    w2_h = nc.dram_tensor("w2", [H, D], f16, kind="ExternalInput")
    w3_h = nc.dram_tensor("w3", [D, H], f16, kind="ExternalInput")
    out_h = nc.dram_tensor("out", [tok, D], f16, kind="ExternalOutput")

    with tile.TileContext(nc) as tc:
        with (
            tc.tile_pool(name="wpool", bufs=1) as wpool,
            tc.tile_pool(name="xtpool", bufs=4) as xtpool,
            tc.tile_pool(name="htpool", bufs=2) as htpool,
            tc.tile_pool(name="spool", bufs=3) as spool,
            tc.tile_pool(name="opool", bufs=2) as opool,
            tc.tile_pool(name="pAB", bufs=4, space="PSUM") as pABp,
            tc.tile_pool(name="pC", bufs=4, space="PSUM") as pCp,
        ):
            w1s = wpool.tile([P, n_i, H], f16)
            w3s = wpool.tile([P, n_i, H], f16)
            w2s = wpool.tile([P, n_j, D], f16)

            # ---- startup wave, in priority order: xT block 0 on sync,
            # w1/w3 as per-j 128-col chunks interleaved on gpsimd/scalar
            # (w1 even j on gpsimd, w1 odd j on scalar, then w3 on
            # scalar, w2 on gpsimd).  pA(j) is needed at ~(10+1.7j) us,
            # pB(j) at ~(10+1.7(8+j)) us, mm2 at ~37 us — all chunks
            # land comfortably earlier.
            xT0 = xtpool.tile([P, n_i, TB], f16)
            nc.sync.dma_start(
                out=xT0,
                in_=xT_h[:, 0:TB].rearrange("(c p) t -> p c t", p=P),
            )
            for j in range(n_j):
                eng = nc.gpsimd if j % 2 == 0 else nc.scalar
                eng.dma_start(
                    out=w1s[:, :, j * P:(j + 1) * P],
                    in_=w1_h[:, j * P:(j + 1) * P].rearrange(
                        "(c p) h -> p c h", p=P
                    ),
                )
            for j in range(n_j):
                nc.scalar.dma_start(
                    out=w3s[:, :, j * P:(j + 1) * P],
                    in_=w3_h[:, j * P:(j + 1) * P].rearrange(
                        "(c p) h -> p c h", p=P
                    ),
                )
            nc.gpsimd.dma_start(
                out=w2s,
                in_=w2_h[:, :].rearrange("(c p) h -> p c h", p=P),
            )

            o_r = out_h[:, :].rearrange("(b c p) d -> b p c d", p=P, c=n_tc)

            for b in range(n_blk):
                # ---- xT[p, c, t] = x[b*TB+t, c*128+p], plain DMA
                if b == 0:
                    xT = xT0
                else:
                    xT = xtpool.tile([P, n_i, TB], f16)
                    nc.sync.dma_start(
                        out=xT,
                        in_=xT_h[:, b * TB:(b + 1) * TB].rearrange(
                            "(c p) t -> p c t", p=P
                        ),
                    )

                # ---- mm1/mm3 + SwiGLU -> hT [P(=j in chunk), n_j, TB] fp16
                hT = htpool.tile([P, n_j, TB], f16)
                for j in range(n_j):
                    pA = pABp.tile([P, TB], f32, tag="pAB")
                    pB = pABp.tile([P, TB], f32, tag="pAB")
                    for i in range(n_i):
                        nc.tensor.matmul(
                            pA, w1s[:, i, j * P:(j + 1) * P], xT[:, i, :],
                            start=(i == 0), stop=(i == n_i - 1),
                        )
                    for i in range(n_i):
                        nc.tensor.matmul(
                            pB, w3s[:, i, j * P:(j + 1) * P], xT[:, i, :],
                            start=(i == 0), stop=(i == n_i - 1),
                        )
                    s1 = spool.tile([P, TB], f16)
                    nc.scalar.activation(s1, pA, AF.Silu)
                    nc.vector.tensor_mul(hT[:, j, :], pB, s1)

                # ---- mm2 -> natural-layout out block, drained to fp16.
                # Last block stores per-(t,o) so the final DMA is small.
                o_sb = opool.tile([P, n_tc, D], f16)
                last = b == n_blk - 1
                for t in range(n_tc):
                    for o in range(n_o):
                        pC = pCp.tile([P, 512], f32)
                        for j in range(n_j):
                            nc.tensor.matmul(
                                pC,
                                hT[:, j, t * P:(t + 1) * P],
                                w2s[:, j, o * 512:(o + 1) * 512],
                                start=(j == 0), stop=(j == n_j - 1),
                            )
                        nc.scalar.activation(
                            o_sb[:, t, o * 512:(o + 1) * 512], pC, AF.Copy
                        )
                        if last:
                            nc.scalar.dma_start(
                                out=o_r[b, :, t, o * 512:(o + 1) * 512],
                                in_=o_sb[:, t, o * 512:(o + 1) * 512],
                            )
                    if not last:
                        nc.scalar.dma_start(
                            out=o_r[b, :, t, :], in_=o_sb[:, t, :]
                        )

    nc.compile()
    return nc


def _get_nc(tok):
    if tok not in _CACHE:
        _CACHE[tok] = _build(tok)
    return _CACHE[tok]


def _prep(x, w1, w2, w3, m_sizes):
    """Shared host-side prep: fp16 casts, transpose, sharding, padding.

    Returns (nc, in_maps, sizes)."""
    x = np.asarray(x)
    sizes = np.asarray(m_sizes).astype(np.int64)
    offs = np.concatenate([[0], np.cumsum(sizes)])
    n_exp = sizes.shape[0]
    assert n_exp == N_CORES

    pad = int(max(int(sizes.max()), TB))
    pad = ((pad + TB - 1) // TB) * TB
    nc = _get_nc(pad)

    x16 = x.astype(np.float16)
    w116 = np.asarray(w1).astype(np.float16)
    w216 = np.asarray(w2).astype(np.float16)
    w316 = np.asarray(w3).astype(np.float16)

    in_maps = []
    for e in range(N_CORES):
        xe = x16[offs[e]:offs[e + 1]]
        xeT = np.zeros((D, pad), dtype=np.float16)
        xeT[:, : xe.shape[0]] = xe.T
        in_maps.append({"xT": xeT, "w1": w116[e], "w2": w216[e], "w3": w316[e]})
    return nc, in_maps, sizes


def kernel(x, w1, w2, w3, m_sizes):
    from concourse.bass_utils import run_bass_kernel_spmd

    nc, in_maps, sizes = _prep(x, w1, w2, w3, m_sizes)
    r = run_bass_kernel_spmd(nc, in_maps, core_ids=list(range(N_CORES)))
    out = np.concatenate(
        [r.results[e]["out"][: sizes[e]] for e in range(N_CORES)], axis=0
    )
    return out.astype(np.float32)


# revision 5
# speedup vs baseline: 1.0113x; 1.0113x over previous
"""Grouped SwiGLU expert FFN (MoE) on 8 Trainium2 NeuronCores.

Expert parallelism: expert e's weights + its (pre-sorted) token slice go to
core e. Each core runs x@w1, x@w3, silu/mul, h@w2 for its 8192 tokens.

v8: host-marshalled layouts; all device DMAs are wide-contiguous.
 - x arrives transposed ([dim_in, tokens] fp16) so xT block loads are
   plain 1KB-line DMAs (v5/v6's xbar transposes forced cross-queue
   mode fences; v7's device-side weight rearranges produced 256B-line
   DMAs that crawled under startup contention).
 - w1/w3 arrive as [P, n_j, n_i, 128] (j-chunk-major) and w2 as
   [P, n_j, D]: every weight DMA line is 2-16KB contiguous.  w1 loads
   as two 1MiB halves on gpsimd, w3 the same on scalar, w2 one 2MiB
   transfer on scalar behind w3 (needed only at ~27us).
 - xtpool bufs=2 so only one xT block prefetches during the startup
   wave (v7's 3-block prefetch starved the weight queues).
 - 24 warm-up matmuls on an unwritten SBUF tile run at engine start
   with no deps: they lift the PE's HAM clock gate to 8/8 before the
   first real matmul, removing the ~3.4us cold-clock ramp.
 - fp16 output store (host upcasts); last block drains per-(t,o).

Math per core (dims: t=tokens, i=dim_in, j=dim_hid, o=dim_in):
  mm1/mm3: psum[j,t] += lhsT=w{1,3}[j-chunk][i_chunk] (stationary),
           rhs=xT[i_chunk, t_block] (moving 512) -> h1T/h3T.
  SwiGLU:  hT = silu(h1T) * h3T  (ACT Silu -> fp16, DVE mul -> fp16).
  mm2:     lhsT=hT[j_chunk, t_chunk] (stationary), rhs=w2[j_chunk, o_block]
           (moving 512) -> psum[t,o] natural-layout output -> fp16 store.
"""

import sys

sys.path.insert(0, "/opt/trn_rl_repo")

import numpy as np

N_CORES = 8
D = 1024  # dim_in
H = 1024  # dim_hid
P = 128
TB = 512  # token block per pipeline stage
N_WARM = 24  # warm-up matmuls to defeat the HAM cold clock

_CACHE = {}


def _build(tok):
    import concourse.bacc as bacc
    import concourse.tile as tile
    from concourse import mybir

    dt = mybir.dt
    AF = mybir.ActivationFunctionType
    f32 = dt.float32
    f16 = dt.float16

    assert tok % TB == 0
    n_blk = tok // TB
    n_i = D // P   # 8 contraction chunks for mm1/mm3
    n_j = H // P   # 8 contraction chunks for mm2
    n_tc = TB // P  # 4 token chunks per block
    n_o = D // 512  # 2 output column blocks

    nc = bacc.Bacc(trn_type="TRN2", target_bir_lowering=False)
    xT_h = nc.dram_tensor("xT", [D, tok], f16, kind="ExternalInput")
    # w1/w3 pre-rearranged on host: [p, j, i, h'] = w[i*128+p, j*128+h']
    w1_h = nc.dram_tensor("w1r", [P, n_j, n_i, P], f16, kind="ExternalInput")
    w3_h = nc.dram_tensor("w3r", [P, n_j, n_i, P], f16, kind="ExternalInput")
    # w2 pre-rearranged on host: [p, j, d] = w2[j*128+p, d]
    w2_h = nc.dram_tensor("w2r", [P, n_j, D], f16, kind="ExternalInput")
    out_h = nc.dram_tensor("out", [tok, D], f16, kind="ExternalOutput")

    with tile.TileContext(nc) as tc:
        with (
            tc.tile_pool(name="wpool", bufs=1) as wpool,
            tc.tile_pool(name="xtpool", bufs=2) as xtpool,
            tc.tile_pool(name="htpool", bufs=2) as htpool,
            tc.tile_pool(name="spool", bufs=3) as spool,
            tc.tile_pool(name="opool", bufs=2) as opool,
            tc.tile_pool(name="pAB", bufs=4, space="PSUM") as pABp,
            tc.tile_pool(name="pC", bufs=4, space="PSUM") as pCp,
        ):
            # ---- PE warm-up: matmuls on a vector-memset tile.  They
            # issue right after the DVE memset (~2.5us), hold the PE
            # busy past the HAM activity window, and finish just before
            # the first real matmul is ready (~10us), which then runs
            # at the full 2.4 GHz instead of paying the cold-clock ramp.
            warm = spool.tile([P, TB], f16)
            nc.vector.memset(warm, 0.0)
            pW = pABp.tile([P, TB], f32, tag="pAB")
            for k in range(N_WARM):
                nc.tensor.matmul(
                    pW, warm[:, 0:P], warm,
                    start=(k == 0), stop=(k == N_WARM - 1),
                )

            w1s = wpool.tile([P, n_j, n_i, P], f16)
            w3s = wpool.tile([P, n_j, n_i, P], f16)
            w2s = wpool.tile([P, n_j, D], f16)

            # ---- startup wave: xT0 on sync; w1 halves on gpsimd; w3
            # halves + w2 on scalar.  All lines are >=2KB contiguous.
            xT0 = xtpool.tile([P, n_i, TB], f16)
            nc.sync.dma_start(
                out=xT0,
                in_=xT_h[:, 0:TB].rearrange("(c p) t -> p c t", p=P),
            )
            half = n_j // 2
            nc.gpsimd.dma_start(out=w1s[:, 0:half], in_=w1_h[:, 0:half])
            nc.scalar.dma_start(out=w3s[:, 0:half], in_=w3_h[:, 0:half])
            nc.gpsimd.dma_start(out=w1s[:, half:], in_=w1_h[:, half:])
            nc.scalar.dma_start(out=w3s[:, half:], in_=w3_h[:, half:])
            nc.scalar.dma_start(out=w2s, in_=w2_h[:, :])

            o_r = out_h[:, :].rearrange("(b c p) d -> b p c d", p=P, c=n_tc)

            for b in range(n_blk):
                # ---- xT[p, c, t] = x[b*TB+t, c*128+p], plain DMA
                if b == 0:
                    xT = xT0
                else:
                    xT = xtpool.tile([P, n_i, TB], f16)
                    nc.sync.dma_start(
                        out=xT,
                        in_=xT_h[:, b * TB:(b + 1) * TB].rearrange(
                            "(c p) t -> p c t", p=P
                        ),
                    )

                # ---- mm1/mm3 + SwiGLU -> hT [P(=j in chunk), n_j, TB] fp16
                hT = htpool.tile([P, n_j, TB], f16)
                for j in range(n_j):
                    pA = pABp.tile([P, TB], f32, tag="pAB")
                    pB = pABp.tile([P, TB], f32, tag="pAB")
                    for i in range(n_i):
                        nc.tensor.matmul(
                            pA, w1s[:, j, i, :], xT[:, i, :],
                            start=(i == 0), stop=(i == n_i - 1),
                        )
                    for i in range(n_i):
                        nc.tensor.matmul(
                            pB, w3s[:, j, i, :], xT[:, i, :],
                            start=(i == 0), stop=(i == n_i - 1),
                        )
                    s1 = spool.tile([P, TB], f16)
                    nc.scalar.activation(s1, pA, AF.Silu)
                    nc.vector.tensor_mul(hT[:, j, :], pB, s1)

                # ---- mm2 -> natural-layout out block, drained to fp16.
                # Last block stores per-(t,o) so the final DMA is small.
                o_sb = opool.tile([P, n_tc, D], f16)
                last = b == n_blk - 1
                for t in range(n_tc):
                    for o in range(n_o):
                        pC = pCp.tile([P, 512], f32)
                        for j in range(n_j):
                            nc.tensor.matmul(
                                pC,
                                hT[:, j, t * P:(t + 1) * P],
                                w2s[:, j, o * 512:(o + 1) * 512],
                                start=(j == 0), stop=(j == n_j - 1),
                            )
                        nc.scalar.activation(
                            o_sb[:, t, o * 512:(o + 1) * 512], pC, AF.Copy
                        )
                        if last:
                            nc.scalar.dma_start(
                                out=o_r[b, :, t, o * 512:(o + 1) * 512],
                                in_=o_sb[:, t, o * 512:(o + 1) * 512],
                            )
                    if not last:
                        nc.scalar.dma_start(
                            out=o_r[b, :, t, :], in_=o_sb[:, t, :]
                        )

    nc.compile()
    return nc


def _get_nc(tok):
    if tok not in _CACHE:
        _CACHE[tok] = _build(tok)
    return _CACHE[tok]


def _rearr_w13(w):
    # [D, H] -> [p, j, i, h'] with w[i*128+p, j*128+h']
    return np.ascontiguousarray(
        w.reshape(8, P, 8, P).transpose(1, 2, 0, 3)
    )


def _prep(x, w1, w2, w3, m_sizes):
    """Shared host-side prep: fp16 casts, layout marshalling, padding.

    Returns (nc, in_maps, sizes)."""
    x = np.asarray(x)
    sizes = np.asarray(m_sizes).astype(np.int64)
    offs = np.concatenate([[0], np.cumsum(sizes)])
    n_exp = sizes.shape[0]
    assert n_exp == N_CORES

    pad = int(max(int(sizes.max()), TB))
    pad = ((pad + TB - 1) // TB) * TB
    nc = _get_nc(pad)

    x16 = x.astype(np.float16)
    w116 = np.asarray(w1).astype(np.float16)
    w216 = np.asarray(w2).astype(np.float16)
    w316 = np.asarray(w3).astype(np.float16)

    in_maps = []
    for e in range(N_CORES):
        xe = x16[offs[e]:offs[e + 1]]
        xeT = np.zeros((D, pad), dtype=np.float16)
        xeT[:, : xe.shape[0]] = xe.T
        in_maps.append({
            "xT": xeT,
            "w1r": _rearr_w13(w116[e]),
            "w3r": _rearr_w13(w316[e]),
            "w2r": np.ascontiguousarray(
                w216[e].reshape(8, P, D).transpose(1, 0, 2)
            ),
        })
    return nc, in_maps, sizes


def kernel(x, w1, w2, w3, m_sizes):
    from concourse.bass_utils import run_bass_kernel_spmd

    nc, in_maps, sizes = _prep(x, w1, w2, w3, m_sizes)
    r = run_bass_kernel_spmd(nc, in_maps, core_ids=list(range(N_CORES)))
    out = np.concatenate(
        [r.results[e]["out"][: sizes[e]] for e in range(N_CORES)], axis=0
    )
    return out.astype(np.float32)


# revision 6
# speedup vs baseline: 1.0402x; 1.0285x over previous
"""Grouped SwiGLU expert FFN (MoE) on 8 Trainium2 NeuronCores.

Expert parallelism: expert e's weights + its (pre-sorted) token slice go to
core e. Each core runs x@w1, x@w3, silu/mul, h@w2 for its 8192 tokens.

v8: host-marshalled layouts; all device DMAs are wide-contiguous.
 - x arrives transposed ([dim_in, tokens] fp16) so xT block loads are
   plain 1KB-line DMAs (v5/v6's xbar transposes forced cross-queue
   mode fences; v7's device-side weight rearranges produced 256B-line
   DMAs that crawled under startup contention).
 - w1/w3 arrive as [P, n_j, n_i, 128] (j-chunk-major) and w2 as
   [P, n_j, D]: every weight DMA line is 2-16KB contiguous.  w1 loads
   as two 1MiB halves on gpsimd, w3 the same on scalar, w2 one 2MiB
   transfer on scalar behind w3 (needed only at ~27us).
 - xtpool bufs=2 so only one xT block prefetches during the startup
   wave (v7's 3-block prefetch starved the weight queues).
 - 24 warm-up matmuls on an unwritten SBUF tile run at engine start
   with no deps: they lift the PE's HAM clock gate to 8/8 before the
   first real matmul, removing the ~3.4us cold-clock ramp.
 - fp16 output store (host upcasts); last block drains per-(t,o).

Math per core (dims: t=tokens, i=dim_in, j=dim_hid, o=dim_in):
  mm1/mm3: psum[j,t] += lhsT=w{1,3}[j-chunk][i_chunk] (stationary),
           rhs=xT[i_chunk, t_block] (moving 512) -> h1T/h3T.
  SwiGLU:  hT = silu(h1T) * h3T  (ACT Silu -> fp16, DVE mul -> fp16).
  mm2:     lhsT=hT[j_chunk, t_chunk] (stationary), rhs=w2[j_chunk, o_block]
           (moving 512) -> psum[t,o] natural-layout output -> fp16 store.
"""

import sys

sys.path.insert(0, "/opt/trn_rl_repo")

import numpy as np

N_CORES = 8
D = 1024  # dim_in
H = 1024  # dim_hid
P = 128
TB = 512  # token block per pipeline stage
N_WARM = 24  # warm-up matmuls to defeat the HAM cold clock

_CACHE = {}


def _build(tok):
    import concourse.bacc as bacc
    import concourse.tile as tile
    from concourse import mybir

    dt = mybir.dt
    AF = mybir.ActivationFunctionType
    f32 = dt.float32
    f16 = dt.float16

    assert tok % TB == 0
    n_blk = tok // TB
    n_i = D // P   # 8 contraction chunks for mm1/mm3
    n_j = H // P   # 8 contraction chunks for mm2
    n_tc = TB // P  # 4 token chunks per block
    n_o = D // 512  # 2 output column blocks

    nc = bacc.Bacc(trn_type="TRN2", target_bir_lowering=False)
    xT_h = nc.dram_tensor("xT", [D, tok], f16, kind="ExternalInput")
    # w1/w3 pre-rearranged on host: [p, j, i, h'] = w[i*128+p, j*128+h']
    w1_h = nc.dram_tensor("w1r", [P, n_j, n_i, P], f16, kind="ExternalInput")
    w3_h = nc.dram_tensor("w3r", [P, n_j, n_i, P], f16, kind="ExternalInput")
    # w2 pre-rearranged on host: [p, j, d] = w2[j*128+p, d]
    w2_h = nc.dram_tensor("w2r", [P, n_j, D], f16, kind="ExternalInput")
    out_h = nc.dram_tensor("out", [tok, D], f16, kind="ExternalOutput")

    with tile.TileContext(nc) as tc:
        with (
            tc.tile_pool(name="wpool", bufs=1) as wpool,
            tc.tile_pool(name="xtpool", bufs=2) as xtpool,
            tc.tile_pool(name="htpool", bufs=2) as htpool,
            tc.tile_pool(name="spool", bufs=3) as spool,
            tc.tile_pool(name="opool", bufs=2) as opool,
            tc.tile_pool(name="pAB", bufs=4, space="PSUM") as pABp,
            tc.tile_pool(name="pC", bufs=4, space="PSUM") as pCp,
        ):
            # ---- PE warm-up: matmuls on a vector-memset tile.  They
            # issue right after the DVE memset (~2.5us), hold the PE
            # busy past the HAM activity window, and finish just before
            # the first real matmul is ready (~10us), which then runs
            # at the full 2.4 GHz instead of paying the cold-clock ramp.
            warm = spool.tile([P, TB], f16)
            nc.vector.memset(warm, 0.0)
            pW = pABp.tile([P, TB], f32, tag="pAB")
            for k in range(N_WARM):
                nc.tensor.matmul(
                    pW, warm[:, 0:P], warm,
                    start=(k == 0), stop=(k == N_WARM - 1),
                )

            w1s = wpool.tile([P, n_j, n_i, P], f16)
            w3s = wpool.tile([P, n_j, n_i, P], f16)
            w2s = wpool.tile([P, n_j, D], f16)

            # ---- startup wave.  The DMA engines share ~360 GB/s, so
            # MM0's critical path must stay skinny: the sync queue
            # carries ONLY xT blocks (xT0 at full bandwidth, ~11us),
            # while every weight load rides gpsimd as a chain of 256KB
            # j-chunks — the SWDGE descriptor builds serialize them,
            # which throttles their bandwidth draw, and each chunk
            # completes long before the PE reaches it.  w2's chunks sit
            # at the end of the chain (first needed ~41us).  The scalar
            # queue carries no startup DMAs at all, so block 0's silu
            # ACTIVATEs issue the moment pA(j=0) is done.
            xT0 = xtpool.tile([P, n_i, TB], f16)
            nc.sync.dma_start(
                out=xT0,
                in_=xT_h[:, 0:TB].rearrange("(c p) t -> p c t", p=P),
            )
            for j in range(n_j):
                nc.gpsimd.dma_start(out=w1s[:, j], in_=w1_h[:, j])
                nc.gpsimd.dma_start(out=w3s[:, j], in_=w3_h[:, j])
            for j in range(n_j):
                nc.gpsimd.dma_start(out=w2s[:, j], in_=w2_h[:, j])

            o_r = out_h[:, :].rearrange("(b c p) d -> b p c d", p=P, c=n_tc)

            for b in range(n_blk):
                # ---- xT[p, c, t] = x[b*TB+t, c*128+p], plain DMA
                if b == 0:
                    xT = xT0
                else:
                    xT = xtpool.tile([P, n_i, TB], f16)
                    nc.sync.dma_start(
                        out=xT,
                        in_=xT_h[:, b * TB:(b + 1) * TB].rearrange(
                            "(c p) t -> p c t", p=P
                        ),
                    )

                # ---- mm1/mm3 + SwiGLU -> hT [P(=j in chunk), n_j, TB] fp16
                hT = htpool.tile([P, n_j, TB], f16)
                for j in range(n_j):
                    pA = pABp.tile([P, TB], f32, tag="pAB")
                    pB = pABp.tile([P, TB], f32, tag="pAB")
                    for i in range(n_i):
                        nc.tensor.matmul(
                            pA, w1s[:, j, i, :], xT[:, i, :],
                            start=(i == 0), stop=(i == n_i - 1),
                        )
                    for i in range(n_i):
                        nc.tensor.matmul(
                            pB, w3s[:, j, i, :], xT[:, i, :],
                            start=(i == 0), stop=(i == n_i - 1),
                        )
                    s1 = spool.tile([P, TB], f16)
                    nc.scalar.activation(s1, pA, AF.Silu)
                    nc.vector.tensor_mul(hT[:, j, :], pB, s1)

                # ---- mm2 -> natural-layout out block, drained to fp16.
                # Last block stores per-(t,o) so the final DMA is small.
                o_sb = opool.tile([P, n_tc, D], f16)
                last = b == n_blk - 1
                for t in range(n_tc):
                    for o in range(n_o):
                        pC = pCp.tile([P, 512], f32)
                        for j in range(n_j):
                            nc.tensor.matmul(
                                pC,
                                hT[:, j, t * P:(t + 1) * P],
                                w2s[:, j, o * 512:(o + 1) * 512],
                                start=(j == 0), stop=(j == n_j - 1),
                            )
                        nc.scalar.activation(
                            o_sb[:, t, o * 512:(o + 1) * 512], pC, AF.Copy
                        )
                        if last:
                            nc.scalar.dma_start(
                                out=o_r[b, :, t, o * 512:(o + 1) * 512],
                                in_=o_sb[:, t, o * 512:(o + 1) * 512],
                            )
                    if not last:
                        nc.scalar.dma_start(
                            out=o_r[b, :, t, :], in_=o_sb[:, t, :]
                        )

    nc.compile()
    return nc


def _get_nc(tok):
    if tok not in _CACHE:
        _CACHE[tok] = _build(tok)
    return _CACHE[tok]


def _rearr_w13(w):
    # [D, H] -> [p, j, i, h'] with w[i*128+p, j*128+h']
    return np.ascontiguousarray(
        w.reshape(8, P, 8, P).transpose(1, 2, 0, 3)
    )


def _prep(x, w1, w2, w3, m_sizes):
    """Shared host-side prep: fp16 casts, layout marshalling, padding.

    Returns (nc, in_maps, sizes)."""
    x = np.asarray(x)
    sizes = np.asarray(m_sizes).astype(np.int64)
    offs = np.concatenate([[0], np.cumsum(sizes)])
    n_exp = sizes.shape[0]
    assert n_exp == N_CORES

    pad = int(max(int(sizes.max()), TB))
    pad = ((pad + TB - 1) // TB) * TB
    nc = _get_nc(pad)

    x16 = x.astype(np.float16)
    w116 = np.asarray(w1).astype(np.float16)
    w216 = np.asarray(w2).astype(np.float16)
    w316 = np.asarray(w3).astype(np.float16)

    in_maps = []
    for e in range(N_CORES):
        xe = x16[offs[e]:offs[e + 1]]
        xeT = np.zeros((D, pad), dtype=np.float16)
        xeT[:, : xe.shape[0]] = xe.T
        in_maps.append({
            "xT": xeT,
            "w1r": _rearr_w13(w116[e]),
            "w3r": _rearr_w13(w316[e]),
            "w2r": np.ascontiguousarray(
                w216[e].reshape(8, P, D).transpose(1, 0, 2)
            ),
        })
    return nc, in_maps, sizes


def kernel(x, w1, w2, w3, m_sizes):
    from concourse.bass_utils import run_bass_kernel_spmd

    nc, in_maps, sizes = _prep(x, w1, w2, w3, m_sizes)
    r = run_bass_kernel_spmd(nc, in_maps, core_ids=list(range(N_CORES)))
    out = np.concatenate(
        [r.results[e]["out"][: sizes[e]] for e in range(N_CORES)], axis=0
    )
    return out.astype(np.float32)


# revision 9
# speedup vs baseline: 1.0407x; 1.0005x over previous
"""Grouped SwiGLU expert FFN (MoE) on 8 Trainium2 NeuronCores.

Expert parallelism: expert e's weights + its (pre-sorted) token slice go to
core e. Each core runs x@w1, x@w3, silu/mul, h@w2 for its 8192 tokens.

v8: host-marshalled layouts; all device DMAs are wide-contiguous.
 - x arrives transposed ([dim_in, tokens] fp16) so xT block loads are
   plain 1KB-line DMAs (v5/v6's xbar transposes forced cross-queue
   mode fences; v7's device-side weight rearranges produced 256B-line
   DMAs that crawled under startup contention).
 - w1/w3 arrive as [P, n_j, n_i, 128] (j-chunk-major) and w2 as
   [P, n_j, D]: every weight DMA line is 2-16KB contiguous.  w1 loads
   as two 1MiB halves on gpsimd, w3 the same on scalar, w2 one 2MiB
   transfer on scalar behind w3 (needed only at ~27us).
 - xtpool bufs=2 so only one xT block prefetches during the startup
   wave (v7's 3-block prefetch starved the weight queues).
 - 24 warm-up matmuls on an unwritten SBUF tile run at engine start
   with no deps: they lift the PE's HAM clock gate to 8/8 before the
   first real matmul, removing the ~3.4us cold-clock ramp.
 - fp16 output store (host upcasts); last block drains per-(t,o).

Math per core (dims: t=tokens, i=dim_in, j=dim_hid, o=dim_in):
  mm1/mm3: psum[j,t] += lhsT=w{1,3}[j-chunk][i_chunk] (stationary),
           rhs=xT[i_chunk, t_block] (moving 512) -> h1T/h3T.
  SwiGLU:  hT = silu(h1T) * h3T  (ACT Silu -> fp16, DVE mul -> fp16).
  mm2:     lhsT=hT[j_chunk, t_chunk] (stationary), rhs=w2[j_chunk, o_block]
           (moving 512) -> psum[t,o] natural-layout output -> fp16 store.
"""

import sys

sys.path.insert(0, "/opt/trn_rl_repo")

import numpy as np

N_CORES = 8
D = 1024  # dim_in
H = 1024  # dim_hid
P = 128
TB = 512  # token block per pipeline stage
N_WARM = 14  # warm-up matmuls to defeat the HAM cold clock

_CACHE = {}


def _build(tok):
    import concourse.bacc as bacc
    import concourse.tile as tile
    from concourse import mybir

    dt = mybir.dt
    AF = mybir.ActivationFunctionType
    f32 = dt.float32
    f16 = dt.float16

    assert tok % TB == 0
    n_blk = tok // TB
    n_i = D // P   # 8 contraction chunks for mm1/mm3
    n_j = H // P   # 8 contraction chunks for mm2
    n_tc = TB // P  # 4 token chunks per block
    n_o = D // 512  # 2 output column blocks

    nc = bacc.Bacc(trn_type="TRN2", target_bir_lowering=False)
    xT_h = nc.dram_tensor("xT", [D, tok], f16, kind="ExternalInput")
    # w1/w3 pre-rearranged on host: [p, j, i, h'] = w[i*128+p, j*128+h']
    w1_h = nc.dram_tensor("w1r", [P, n_j, n_i, P], f16, kind="ExternalInput")
    w3_h = nc.dram_tensor("w3r", [P, n_j, n_i, P], f16, kind="ExternalInput")
    # w2 pre-rearranged on host: [p, j, d] = w2[j*128+p, d]
    w2_h = nc.dram_tensor("w2r", [P, n_j, D], f16, kind="ExternalInput")
    out_h = nc.dram_tensor("out", [tok, D], f16, kind="ExternalOutput")

    with tile.TileContext(nc) as tc:
        with (
            tc.tile_pool(name="wpool", bufs=1) as wpool,
            tc.tile_pool(name="xtpool", bufs=2) as xtpool,
            tc.tile_pool(name="htpool", bufs=2) as htpool,
            tc.tile_pool(name="spool", bufs=3) as spool,
            tc.tile_pool(name="opool", bufs=2) as opool,
            tc.tile_pool(name="pAB", bufs=4, space="PSUM") as pABp,
            tc.tile_pool(name="pC", bufs=4, space="PSUM") as pCp,
        ):
            # ---- PE warm-up: matmuls on a vector-memset tile.  They
            # issue right after the DVE memset (~2.5us), hold the PE
            # busy past the HAM activity window, and finish just before
            # the first real matmul is ready (~10us), which then runs
            # at the full 2.4 GHz instead of paying the cold-clock ramp.
            warm = spool.tile([P, TB], f16)
            nc.vector.memset(warm, 0.0)
            pW = pABp.tile([P, TB], f32, tag="pAB")
            for k in range(N_WARM):
                nc.tensor.matmul(
                    pW, warm[:, 0:P], warm,
                    start=(k == 0), stop=(k == N_WARM - 1),
                )

            w1s = wpool.tile([P, n_j, n_i, P], f16)
            w3s = wpool.tile([P, n_j, n_i, P], f16)
            w2s = wpool.tile([P, n_j, D], f16)

            # ---- startup wave.  The DMA engines share ~360 GB/s, so
            # MM0's critical path must stay skinny: the sync queue
            # carries ONLY xT blocks (xT0 at full bandwidth, ~11us),
            # while every weight load rides gpsimd as a chain of 256KB
            # j-chunks — the SWDGE descriptor builds serialize them,
            # which throttles their bandwidth draw, and each chunk
            # completes long before the PE reaches it.  w2's chunks sit
            # at the end of the chain (first needed ~41us).  The scalar
            # queue carries no startup DMAs at all, so block 0's silu
            # ACTIVATEs issue the moment pA(j=0) is done.
            xT0 = xtpool.tile([P, n_i, TB], f16)
            # xT0 split across the two HWDGE queues: halves land in
            # parallel (~9.5us instead of ~12.5), and the i=0..3 half
            # arrives first, which is all pA(j=0) needs to launch.
            nc.sync.dma_start(
                out=xT0[:, 0: n_i // 2],
                in_=xT_h[0: D // 2, 0:TB].rearrange("(c p) t -> p c t", p=P),
            )
            nc.scalar.dma_start(
                out=xT0[:, n_i // 2:],
                in_=xT_h[D // 2:, 0:TB].rearrange("(c p) t -> p c t", p=P),
            )
            for j in range(n_j):
                nc.gpsimd.dma_start(out=w1s[:, j], in_=w1_h[:, j])
                nc.gpsimd.dma_start(out=w3s[:, j], in_=w3_h[:, j])
            for j in range(n_j):
                nc.gpsimd.dma_start(out=w2s[:, j], in_=w2_h[:, j])

            o_r = out_h[:, :].rearrange("(b c p) d -> b p c d", p=P, c=n_tc)

            for b in range(n_blk):
                # ---- xT[p, c, t] = x[b*TB+t, c*128+p], plain DMA
                if b == 0:
                    xT = xT0
                else:
                    xT = xtpool.tile([P, n_i, TB], f16)
                    nc.sync.dma_start(
                        out=xT,
                        in_=xT_h[:, b * TB:(b + 1) * TB].rearrange(
                            "(c p) t -> p c t", p=P
                        ),
                    )

                # ---- mm1/mm3 + SwiGLU -> hT [P(=j in chunk), n_j, TB] fp16
                hT = htpool.tile([P, n_j, TB], f16)
                for j in range(n_j):
                    pA = pABp.tile([P, TB], f32, tag="pAB")
                    pB = pABp.tile([P, TB], f32, tag="pAB")
                    for i in range(n_i):
                        nc.tensor.matmul(
                            pA, w1s[:, j, i, :], xT[:, i, :],
                            start=(i == 0), stop=(i == n_i - 1),
                        )
                    for i in range(n_i):
                        nc.tensor.matmul(
                            pB, w3s[:, j, i, :], xT[:, i, :],
                            start=(i == 0), stop=(i == n_i - 1),
                        )
                    s1 = spool.tile([P, TB], f16)
                    nc.scalar.activation(s1, pA, AF.Silu)
                    nc.vector.tensor_mul(hT[:, j, :], pB, s1)

                # ---- mm2 -> natural-layout out block, drained to fp16.
                # Last block stores per-(t,o) so the final DMA is small.
                o_sb = opool.tile([P, n_tc, D], f16)
                last = b == n_blk - 1
                for t in range(n_tc):
                    for o in range(n_o):
                        pC = pCp.tile([P, 512], f32)
                        for j in range(n_j):
                            nc.tensor.matmul(
                                pC,
                                hT[:, j, t * P:(t + 1) * P],
                                w2s[:, j, o * 512:(o + 1) * 512],
                                start=(j == 0), stop=(j == n_j - 1),
                            )
                        nc.scalar.activation(
                            o_sb[:, t, o * 512:(o + 1) * 512], pC, AF.Copy
                        )
                        if last:
                            # alternate queues so the final stores
                            # overlap and the drain finishes sooner
                            seng = nc.scalar if o == 0 else nc.sync
                            seng.dma_start(
                                out=o_r[b, :, t, o * 512:(o + 1) * 512],
                                in_=o_sb[:, t, o * 512:(o + 1) * 512],
                            )
                    if not last:
                        nc.scalar.dma_start(
                            out=o_r[b, :, t, :], in_=o_sb[:, t, :]
                        )

    nc.compile()
    return nc


def _get_nc(tok):
    if tok not in _CACHE:
        _CACHE[tok] = _build(tok)
    return _CACHE[tok]


def _rearr_w13(w):
    # [D, H] -> [p, j, i, h'] with w[i*128+p, j*128+h']
    return np.ascontiguousarray(
        w.reshape(8, P, 8, P).transpose(1, 2, 0, 3)
    )


def _prep(x, w1, w2, w3, m_sizes):
    """Shared host-side prep: fp16 casts, layout marshalling, padding.

    Returns (nc, in_maps, sizes)."""
    x = np.asarray(x)
    sizes = np.asarray(m_sizes).astype(np.int64)
    offs = np.concatenate([[0], np.cumsum(sizes)])
    n_exp = sizes.shape[0]
    assert n_exp == N_CORES

    pad = int(max(int(sizes.max()), TB))
    pad = ((pad + TB - 1) // TB) * TB
    nc = _get_nc(pad)

    x16 = x.astype(np.float16)
    w116 = np.asarray(w1).astype(np.float16)
    w216 = np.asarray(w2).astype(np.float16)
    w316 = np.asarray(w3).astype(np.float16)

    in_maps = []
    for e in range(N_CORES):
        xe = x16[offs[e]:offs[e + 1]]
        xeT = np.zeros((D, pad), dtype=np.float16)
        xeT[:, : xe.shape[0]] = xe.T
        in_maps.append({
            "xT": xeT,
            "w1r": _rearr_w13(w116[e]),
            "w3r": _rearr_w13(w316[e]),
            "w2r": np.ascontiguousarray(
                w216[e].reshape(8, P, D).transpose(1, 0, 2)
            ),
        })
    return nc, in_maps, sizes


def kernel(x, w1, w2, w3, m_sizes):
    from concourse.bass_utils import run_bass_kernel_spmd

    nc, in_maps, sizes = _prep(x, w1, w2, w3, m_sizes)
    r = run_bass_kernel_spmd(nc, in_maps, core_ids=list(range(N_CORES)))
    out = np.concatenate(
        [r.results[e]["out"][: sizes[e]] for e in range(N_CORES)], axis=0
    )
    return out.astype(np.float32)
